# revision 28
# baseline (speedup 1.0000x reference)
"""BigBird block-sparse attention on 8 Trainium2 NeuronCores.

Problem: B=2, H=16, F=T=1024, D=64, 64x64-block BigBird mask per head.
  scores = (Q @ K^T) / 8 + (1-mask) * -10000
  out    = softmax(scores) @ V, laid out [B, F, H, D]

Sharding: head-parallel. Core c handles heads {2c, 2c+1} x both batches
= 4 (b,h) pairs; no cross-core communication.

Kernel design (per (b,h) pair):
  * Compute S^T [t, f] = K Q^T so the P@V matmul needs no transposes:
    PV uses lhsT = P^T tiles (natural layout) and rhs = V (natural).
  * The mask is block-constant over 64-row from-blocks, so the additive
    bias (1-mask)*-10000 is rank-16 along f: bias(t, f) = B[fb(f), t]
    with fb(f) = f // 64.  We fold it into the QK^T matmul by extending
    the contraction dim from 64 to 80:
       lhsT rows 64:80 = B[c, t]          (per head, from the mask)
       rhs  rows 64:80 = onehot[c, f]     (constant)
    so scores come out of the PE already masked - zero extra instructions
    and the 64MB f32 mask becomes 64KB per head of bias rows.
  * No max-subtraction in softmax: scores ~ N(0,1) (|s| < ~7), exp can't
    overflow f32, and masked lanes get exp(s - ~1e4) == 0.0 exactly, same
    as the reference (which also underflows to exact zero in f32).
  * Softmax denominator: V gets a ones-column appended ([t, 65]); the PV
    accumulation then yields rowsum in column 64 for free.  One DVE
    reciprocal + one broadcast tensor_tensor multiply normalizes.

dtype: fp16 matmul inputs (f32 PSUM accumulate), f32 output.
"""

import numpy as np

B, H, F, T, D = 2, 16, 1024, 1024, 64
BS = 64                  # mask block size
NB = F // BS             # 16 from-blocks
KEXT = D + NB            # 80: contraction dim with bias rows folded in
N_CORES = 8
HEADS_PER_CORE = H // N_CORES          # 2
PAIRS = HEADS_PER_CORE * B             # 4 (b,h) pairs per core
NEG = -10000.0

_CACHE = {}


def _build_nc():
    """Build + finalize the per-core Bass program (identical on all cores)."""
    import concourse.tile as tile
    from concourse import bacc, mybir

    nc = bacc.Bacc(None, target_bir_lowering=False)
    f16 = mybir.dt.float16
    f32 = mybir.dt.float32

    kte = nc.dram_tensor("kte", [PAIRS, KEXT, T], f16, kind="ExternalInput")
    qte = nc.dram_tensor("qte", [PAIRS, KEXT, F], f16, kind="ExternalInput")
    v1 = nc.dram_tensor("v1", [PAIRS, 128, 8 * 65], f16, kind="ExternalInput")
    out = nc.dram_tensor("out", [PAIRS, F, D], f32, kind="ExternalOutput")

    Exp = mybir.ActivationFunctionType.Exp

    with tile.TileContext(nc) as tc:
        with (
            tc.tile_pool(name="io", bufs=2) as io_pool,
            tc.tile_pool(name="pt", bufs=6) as pt_pool,
            tc.tile_pool(name="res", bufs=4) as res_pool,
            tc.tile_pool(name="spsum", bufs=2, space="PSUM") as s_psum,
            tc.tile_pool(name="opsum", bufs=2, space="PSUM") as o_psum,
        ):
            # Score columns flat-packed [tb, fc] -> c = tb*1024 + fc*512 into
            # PSUM tiles of width 512,1536x5 (exp is elementwise, so one op
            # can span several t-blocks; wider ops amortize the per-op ACT
            # access bubble).  All 512-chunk and 128-col PV slices stay
            # inside one tile and one bank.
            widths = [512] + [1536] * 5
            starts = np.cumsum([0] + widths).tolist()
            state = {}  # per-pair emission state

            def emit_front(p, ti):
                """QK matmuls + exp for stage (pair p, tile ti)."""
                if ti == 0:
                    kt = io_pool.tile([KEXT, T], f16, tag="kt")
                    qt = io_pool.tile([KEXT, F], f16, tag="qt")
                    vt = io_pool.tile([128, 8 * 65], f16, tag="vt")
                    # split loads across the sync + gpsimd DMA queues
                    nc.sync.dma_start(out=kt[:, 0:512], in_=kte[p, :, 0:512])
                    nc.gpsimd.dma_start(out=kt[:, 512:], in_=kte[p, :, 512:])
                    nc.sync.dma_start(out=qt[:, 0:512], in_=qte[p, :, 0:512])
                    nc.gpsimd.dma_start(out=qt[:, 512:], in_=qte[p, :, 512:])
                    nc.gpsimd.dma_start(out=vt[:], in_=v1[p, :, :])
                    # PV accumulators: 4 f-blocks packed per PSUM bank
                    # (65 cols each: 64 out + rowsum); one accumulation
                    # group per bank, rotating through a 2-buf pool tag.
                    o_lo = o_psum.tile([128, 4 * 65], f32, tag="o")
                    o_hi = o_psum.tile([128, 4 * 65], f32, tag="o")
                    state[p] = dict(kt=kt, qt=qt, vt=vt, o_lo=o_lo, o_hi=o_hi,
                                    pts=[])
                st = state[p]
                w = widths[ti]
                s_ps = s_psum.tile([128, w], f32, tag="s")
                for off in range(0, w, 512):
                    c = starts[ti] + off
                    tb, fc = c // 1024, (c % 1024) // 512
                    nc.tensor.matmul(
                        s_ps[:, off : off + 512],
                        lhsT=st["kt"][:, tb * 128 : (tb + 1) * 128],
                        rhs=st["qt"][:, fc * 512 : (fc + 1) * 512],
                        start=True,
                        stop=True,
                    )
                pt = pt_pool.tile([128, w], f16, tag="p")
                st["pts"].append(pt)
                if p == PAIRS - 1 and ti == 5:
                    # split the very last exp so the lo-phase PV/normalize/
                    # store overlaps the final 512-col exp (shorter tail)
                    nc.scalar.activation(pt[:, 0:1024], s_ps[:, 0:1024], Exp)
                    nc.scalar.activation(pt[:, 1024:], s_ps[:, 1024:], Exp)
                else:
                    nc.scalar.activation(pt[:], s_ps[:], Exp)

            def emit_pv(p, ti):
                """PV matmuls consuming tile ti of pair p, then (after the
                last tile) the normalize + store."""
                st = state[p]
                for c in range(starts[ti], starts[ti + 1], 128):
                    tb, fo = c // 1024, (c % 1024) // 128
                    off = c - starts[ti]
                    o_ps = st["o_lo"] if fo < 4 else st["o_hi"]
                    fi = fo % 4
                    nc.tensor.matmul(
                        o_ps[:, fi * 65 : (fi + 1) * 65],
                        lhsT=st["pts"][ti][:, off : off + 128],
                        rhs=st["vt"][:, tb * 65 : (tb + 1) * 65],
                        start=(tb == 0 and fi == 0),
                        stop=(tb == 7 and fi == 3),
                    )
                if ti < 5:
                    return
                o_sb = res_pool.tile([128, 8, D], f32, tag="os")
                recip = res_pool.tile([128, 8], f32, tag="r")
                out_r = out[p, :, :].rearrange("(fo ti) d -> ti fo d", ti=128)
                for half, o_ps in ((0, st["o_lo"]), (1, st["o_hi"])):
                    rc = recip[:, half * 4 : half * 4 + 4]
                    nc.vector.reciprocal(rc, o_ps[:, 64 :: 65])
                    # one broadcast multiply per half: strided in0 skips the
                    # rowsum columns; in1 broadcasts each recip across D cols
                    nc.vector.tensor_mul(
                        o_sb[:, half * 4 : half * 4 + 4, :],
                        o_ps[:].rearrange("p (fi c) -> p fi c", c=65)[:, :, 0:64],
                        rc.to_broadcast([128, 4, 64]),
                    )
                    # half-pair DMA out: sbuf [t_in,fo,d] -> dram [fo,t_in,d];
                    # lo on gpsimd, hi on sync so the two inits overlap
                    eng = nc.gpsimd if half == 0 else nc.sync
                    eng.dma_start(
                        out=out_r[:, half * 4 : half * 4 + 4, :],
                        in_=o_sb[:, half * 4 : half * 4 + 4, :],
                    )
                del state[p]

            # Software-pipelined emission: PV lags LAG stages behind QK/exp.
            # PE's runtime queue is FIFO, so an o-accumulator-blocked PV
            # matmul emitted too early would head-of-line-block the next
            # pair's QK matmuls and starve ACT at pair boundaries.
            stages = [(p, ti) for p in range(PAIRS) for ti in range(6)]
            pv_next = 0
            for s, (p, ti) in enumerate(stages):
                emit_front(p, ti)
                # taper the lag to 1 inside the last pair so less PV work
                # trails the final exp (shorter kernel tail)
                lag = 1 if p == PAIRS - 1 else 2
                while pv_next <= s - lag:
                    emit_pv(*stages[pv_next])
                    pv_next += 1
            while pv_next < len(stages):
                emit_pv(*stages[pv_next])
                pv_next += 1

    nc.finalize()
    return nc


def _prep_inputs(query_layer, key_layer, value_layer, attention_mask):
    """Host-side shard prep: per-core input maps."""
    bf = np.float16
    q = np.asarray(query_layer, dtype=np.float32)
    k = np.asarray(key_layer, dtype=np.float32)
    v = np.asarray(value_layer, dtype=np.float32)
    m = np.asarray(attention_mask, dtype=np.float32)

    # The BigBird mask is constant within each 64-row from-block; sample one
    # row per block (validated cheaply on a few offsets).
    mrow = m[0, :, ::BS, :]                      # [H, NB, T]
    for off in (17, 63):
        if not np.array_equal(mrow, m[0, :, off::BS, :]):
            raise ValueError("mask not constant within 64-row blocks")
    bias = (mrow - 1.0) * (-NEG)                 # 0 where attending, -1e4 where masked
    bias_bf = bias.astype(bf)                    # [H, NB, T]

    onehot = np.zeros((NB, F), dtype=bf)
    for c in range(NB):
        onehot[c, c * BS : (c + 1) * BS] = 1.0

    qT = (q * 0.125).transpose(0, 1, 3, 2).astype(bf)   # [B, H, D, F]
    kT = k.transpose(0, 1, 3, 2).astype(bf)             # [B, H, D, T]
    v1 = np.concatenate(
        [v.astype(bf), np.ones((B, H, T, 1), dtype=bf)], axis=-1
    )                                                   # [B, H, T, 65]
    # [t_in 128, tb 8, 65] per pair, flattened free dim
    v1r = v1.reshape(B, H, 8, 128, 65).transpose(0, 1, 3, 2, 4).reshape(B, H, 128, 8 * 65)

    in_maps = []
    pair_index = []  # (b, h) per pair slot, per core
    for c in range(N_CORES):
        kte = np.empty((PAIRS, KEXT, T), dtype=bf)
        qte = np.empty((PAIRS, KEXT, F), dtype=bf)
        v1c = np.empty((PAIRS, 128, 8 * 65), dtype=bf)
        pairs = []
        for p in range(PAIRS):
            h = HEADS_PER_CORE * c + p // B
            b = p % B
            kte[p, :D] = kT[b, h]
            kte[p, D:] = bias_bf[h]
            qte[p, :D] = qT[b, h]
            qte[p, D:] = onehot
            v1c[p] = v1r[b, h]
            pairs.append((b, h))
        in_maps.append({"kte": kte, "qte": qte, "v1": v1c})
        pair_index.append(pairs)
    return in_maps, pair_index


def kernel(query_layer, key_layer, value_layer, attention_mask):
    from concourse.bass_utils import run_bass_kernel_spmd

    if "nc" not in _CACHE:
        _CACHE["nc"] = _build_nc()
    nc = _CACHE["nc"]

    in_maps, pair_index = _prep_inputs(
        query_layer, key_layer, value_layer, attention_mask
    )
    core_ids = list(range(N_CORES))
    try:
        res = run_bass_kernel_spmd(nc, in_maps, core_ids)
    except Exception:
        # transient device errors (e.g. NRT_EXEC_UNIT_UNRECOVERABLE) clear
        # on redispatch
        res = run_bass_kernel_spmd(nc, in_maps, core_ids)

    out = np.empty((B, F, H, D), dtype=np.float32)
    for c in range(N_CORES):
        core_out = res.results[c]["out"]         # [PAIRS, F, D]
        for p, (b, h) in enumerate(pair_index[c]):
            out[b, :, h, :] = core_out[p]
    return out


# revision 29
# speedup vs baseline: 1.0081x; 1.0081x over previous
"""BigBird block-sparse attention on 8 Trainium2 NeuronCores.

Problem: B=2, H=16, F=T=1024, D=64, 64x64-block BigBird mask per head.
  scores = (Q @ K^T) / 8 + (1-mask) * -10000
  out    = softmax(scores) @ V, laid out [B, F, H, D]

Sharding: head-parallel. Core c handles heads {2c, 2c+1} x both batches
= 4 (b,h) pairs; no cross-core communication.

Kernel design (per (b,h) pair):
  * Compute S^T [t, f] = K Q^T so the P@V matmul needs no transposes:
    PV uses lhsT = P^T tiles (natural layout) and rhs = V (natural).
  * The mask is block-constant over 64-row from-blocks, so the additive
    bias (1-mask)*-10000 is rank-16 along f: bias(t, f) = B[fb(f), t]
    with fb(f) = f // 64.  We fold it into the QK^T matmul by extending
    the contraction dim from 64 to 80:
       lhsT rows 64:80 = B[c, t]          (per head, from the mask)
       rhs  rows 64:80 = onehot[c, f]     (constant)
    so scores come out of the PE already masked - zero extra instructions
    and the 64MB f32 mask becomes 64KB per head of bias rows.
  * No max-subtraction in softmax: scores ~ N(0,1) (|s| < ~7), exp can't
    overflow f32, and masked lanes get exp(s - ~1e4) == 0.0 exactly, same
    as the reference (which also underflows to exact zero in f32).
  * Softmax denominator: V gets a ones-column appended ([t, 65]); the PV
    accumulation then yields rowsum in column 64 for free.  One DVE
    reciprocal + one broadcast tensor_tensor multiply normalizes.

dtype: fp16 matmul inputs (f32 PSUM accumulate), f32 output.
"""

import numpy as np

B, H, F, T, D = 2, 16, 1024, 1024, 64
BS = 64                  # mask block size
NB = F // BS             # 16 from-blocks
KEXT = D + NB            # 80: contraction dim with bias rows folded in
N_CORES = 8
HEADS_PER_CORE = H // N_CORES          # 2
PAIRS = HEADS_PER_CORE * B             # 4 (b,h) pairs per core
NEG = -10000.0

_CACHE = {}


def _build_nc():
    """Build + finalize the per-core Bass program (identical on all cores)."""
    import concourse.tile as tile
    from concourse import bacc, mybir

    nc = bacc.Bacc(None, target_bir_lowering=False)
    f16 = mybir.dt.float16
    f32 = mybir.dt.float32

    kte = nc.dram_tensor("kte", [PAIRS, KEXT, T], f16, kind="ExternalInput")
    qte = nc.dram_tensor("qte", [PAIRS, KEXT, F], f16, kind="ExternalInput")
    v1 = nc.dram_tensor("v1", [PAIRS, 128, 8 * 65], f16, kind="ExternalInput")
    out = nc.dram_tensor("out", [PAIRS, F, D], f32, kind="ExternalOutput")

    Exp = mybir.ActivationFunctionType.Exp

    with tile.TileContext(nc) as tc:
        with (
            tc.tile_pool(name="io", bufs=2) as io_pool,
            tc.tile_pool(name="pt", bufs=6) as pt_pool,
            tc.tile_pool(name="res", bufs=4) as res_pool,
            tc.tile_pool(name="spsum", bufs=2, space="PSUM") as s_psum,
            tc.tile_pool(name="opsum", bufs=2, space="PSUM") as o_psum,
        ):
            # Score columns flat-packed [tb, fc] -> c = tb*1024 + fc*512 into
            # PSUM tiles of width 512,1536x5 (exp is elementwise, so one op
            # can span several t-blocks; wider ops amortize the per-op ACT
            # access bubble).  All 512-chunk and 128-col PV slices stay
            # inside one tile and one bank.
            widths = [512] + [1536] * 5
            starts = np.cumsum([0] + widths).tolist()
            state = {}  # per-pair emission state

            def emit_front(p, ti):
                """QK matmuls + exp for stage (pair p, tile ti)."""
                if ti == 0:
                    kt = io_pool.tile([KEXT, T], f16, tag="kt")
                    qt = io_pool.tile([KEXT, F], f16, tag="qt")
                    vt = io_pool.tile([128, 8 * 65], f16, tag="vt")
                    # split loads across the sync + gpsimd DMA queues
                    nc.sync.dma_start(out=kt[:, 0:512], in_=kte[p, :, 0:512])
                    nc.gpsimd.dma_start(out=kt[:, 512:], in_=kte[p, :, 512:])
                    nc.sync.dma_start(out=qt[:, 0:512], in_=qte[p, :, 0:512])
                    nc.gpsimd.dma_start(out=qt[:, 512:], in_=qte[p, :, 512:])
                    nc.gpsimd.dma_start(out=vt[:], in_=v1[p, :, :])
                    # PV accumulators: 4 f-blocks packed per PSUM bank
                    # (65 cols each: 64 out + rowsum); one accumulation
                    # group per bank, rotating through a 2-buf pool tag.
                    o_lo = o_psum.tile([128, 4 * 65], f32, tag="o")
                    o_hi = o_psum.tile([128, 4 * 65], f32, tag="o")
                    state[p] = dict(kt=kt, qt=qt, vt=vt, o_lo=o_lo, o_hi=o_hi,
                                    pts=[])
                st = state[p]
                w = widths[ti]
                s_ps = s_psum.tile([128, w], f32, tag="s")
                for off in range(0, w, 512):
                    c = starts[ti] + off
                    tb, fc = c // 1024, (c % 1024) // 512
                    nc.tensor.matmul(
                        s_ps[:, off : off + 512],
                        lhsT=st["kt"][:, tb * 128 : (tb + 1) * 128],
                        rhs=st["qt"][:, fc * 512 : (fc + 1) * 512],
                        start=True,
                        stop=True,
                    )
                pt = pt_pool.tile([128, w], f16, tag="p")
                st["pts"].append(pt)
                if p == PAIRS - 1 and ti == 5:
                    # split the very last exp so the lo-phase PV/normalize/
                    # store overlaps the final 512-col exp (shorter tail)
                    nc.scalar.activation(pt[:, 0:1024], s_ps[:, 0:1024], Exp)
                    nc.scalar.activation(pt[:, 1024:], s_ps[:, 1024:], Exp)
                else:
                    nc.scalar.activation(pt[:], s_ps[:], Exp)

            def emit_pv(p, ti):
                """PV matmuls consuming tile ti of pair p, then (after the
                last tile) the normalize + store."""
                st = state[p]
                for c in range(starts[ti], starts[ti + 1], 128):
                    tb, fo = c // 1024, (c % 1024) // 128
                    off = c - starts[ti]
                    o_ps = st["o_lo"] if fo < 4 else st["o_hi"]
                    fi = fo % 4
                    nc.tensor.matmul(
                        o_ps[:, fi * 65 : (fi + 1) * 65],
                        lhsT=st["pts"][ti][:, off : off + 128],
                        rhs=st["vt"][:, tb * 65 : (tb + 1) * 65],
                        start=(tb == 0 and fi == 0),
                        stop=(tb == 7 and fi == 3),
                    )
                if ti < 5:
                    return
                o_sb = res_pool.tile([128, 8, D], f32, tag="os")
                recip = res_pool.tile([128, 8], f32, tag="r")
                out_r = out[p, :, :].rearrange("(fo ti) d -> ti fo d", ti=128)
                for half, o_ps in ((0, st["o_lo"]), (1, st["o_hi"])):
                    rc = recip[:, half * 4 : half * 4 + 4]
                    nc.vector.reciprocal(rc, o_ps[:, 64 :: 65])
                    # one broadcast multiply per half: strided in0 skips the
                    # rowsum columns; in1 broadcasts each recip across D cols
                    nc.vector.tensor_mul(
                        o_sb[:, half * 4 : half * 4 + 4, :],
                        o_ps[:].rearrange("p (fi c) -> p fi c", c=65)[:, :, 0:64],
                        rc.to_broadcast([128, 4, 64]),
                    )
                    # half-pair DMA out: sbuf [t_in,fo,d] -> dram [fo,t_in,d];
                    # lo on gpsimd, hi on sync so the two inits overlap
                    eng = nc.gpsimd if half == 0 else nc.sync
                    eng.dma_start(
                        out=out_r[:, half * 4 : half * 4 + 4, :],
                        in_=o_sb[:, half * 4 : half * 4 + 4, :],
                    )
                del state[p]

            # Software-pipelined emission: PV lags LAG stages behind QK/exp.
            # PE's runtime queue is FIFO, so an o-accumulator-blocked PV
            # matmul emitted too early would head-of-line-block the next
            # pair's QK matmuls and starve ACT at pair boundaries.
            stages = [(p, ti) for p in range(PAIRS) for ti in range(6)]
            pv_next = 0
            for s, (p, ti) in enumerate(stages):
                emit_front(p, ti)
                lag = 2
                while pv_next <= s - lag:
                    emit_pv(*stages[pv_next])
                    pv_next += 1
            while pv_next < len(stages):
                emit_pv(*stages[pv_next])
                pv_next += 1

    nc.finalize()
    return nc


def _prep_inputs(query_layer, key_layer, value_layer, attention_mask):
    """Host-side shard prep: per-core input maps."""
    bf = np.float16
    q = np.asarray(query_layer, dtype=np.float32)
    k = np.asarray(key_layer, dtype=np.float32)
    v = np.asarray(value_layer, dtype=np.float32)
    m = np.asarray(attention_mask, dtype=np.float32)

    # The BigBird mask is constant within each 64-row from-block; sample one
    # row per block (validated cheaply on a few offsets).
    mrow = m[0, :, ::BS, :]                      # [H, NB, T]
    for off in (17, 63):
        if not np.array_equal(mrow, m[0, :, off::BS, :]):
            raise ValueError("mask not constant within 64-row blocks")
    bias = (mrow - 1.0) * (-NEG)                 # 0 where attending, -1e4 where masked
    bias_bf = bias.astype(bf)                    # [H, NB, T]

    onehot = np.zeros((NB, F), dtype=bf)
    for c in range(NB):
        onehot[c, c * BS : (c + 1) * BS] = 1.0

    qT = (q * 0.125).transpose(0, 1, 3, 2).astype(bf)   # [B, H, D, F]
    kT = k.transpose(0, 1, 3, 2).astype(bf)             # [B, H, D, T]
    v1 = np.concatenate(
        [v.astype(bf), np.ones((B, H, T, 1), dtype=bf)], axis=-1
    )                                                   # [B, H, T, 65]
    # [t_in 128, tb 8, 65] per pair, flattened free dim
    v1r = v1.reshape(B, H, 8, 128, 65).transpose(0, 1, 3, 2, 4).reshape(B, H, 128, 8 * 65)

    in_maps = []
    pair_index = []  # (b, h) per pair slot, per core
    for c in range(N_CORES):
        kte = np.empty((PAIRS, KEXT, T), dtype=bf)
        qte = np.empty((PAIRS, KEXT, F), dtype=bf)
        v1c = np.empty((PAIRS, 128, 8 * 65), dtype=bf)
        pairs = []
        for p in range(PAIRS):
            h = HEADS_PER_CORE * c + p // B
            b = p % B
            kte[p, :D] = kT[b, h]
            kte[p, D:] = bias_bf[h]
            qte[p, :D] = qT[b, h]
            qte[p, D:] = onehot
            v1c[p] = v1r[b, h]
            pairs.append((b, h))
        in_maps.append({"kte": kte, "qte": qte, "v1": v1c})
        pair_index.append(pairs)
    return in_maps, pair_index


def kernel(query_layer, key_layer, value_layer, attention_mask):
    from concourse.bass_utils import run_bass_kernel_spmd

    if "nc" not in _CACHE:
        _CACHE["nc"] = _build_nc()
    nc = _CACHE["nc"]

    in_maps, pair_index = _prep_inputs(
        query_layer, key_layer, value_layer, attention_mask
    )
    core_ids = list(range(N_CORES))
    try:
        res = run_bass_kernel_spmd(nc, in_maps, core_ids)
    except Exception:
        # transient device errors (e.g. NRT_EXEC_UNIT_UNRECOVERABLE) clear
        # on redispatch
        res = run_bass_kernel_spmd(nc, in_maps, core_ids)

    out = np.empty((B, F, H, D), dtype=np.float32)
    for c in range(N_CORES):
        core_out = res.results[c]["out"]         # [PAIRS, F, D]
        for p, (b, h) in enumerate(pair_index[c]):
            out[b, :, h, :] = core_out[p]
    return out
